# revision 38
# baseline (speedup 1.0000x reference)
"""Trainium2 Bass kernel for nn_DecoderLayer (GNN message passing layer).

Data-parallel over the node axis N=4096 across 8 NeuronCores (512
nodes/core). Feature-major compute ([C, rows] in SBUF); edge features are
pre-transposed, cast to bf16 AND laid out k-major (neighbor-major) within
each 32-node super-block on the host. bf16 halves the HBM stream (the
roofline for this memory-bound problem); k-major makes every step of the
k=48 aggregation tree a single fully-contiguous DVE op (strided
sub-row views pay ~55ns per row on the DVE) and makes 512-col m1 groups
node-complete so the stride-0 node-feature matmul aligns with PSUM banks.

Structure per super-block t (1536 edge rows, k-major columns):
  DMA : edges(t+2) one dma_start (128 rows x 9216B)
  PE  : m2(t-1) 3x512 -> ps2; m1(t): per chunk c 3 edge matmuls into
        [pA|pA|pB] + 3 stride-0 node matmuls ([128,16,32] views)
  ACT : gelu2(t-1) one [128,1536] instr; gelu1A [128,1024], gelu1B
        [128,512] (contiguous PSUM, bf16 out)
  GPS : attention row broadcast (from one whole-run attn DMA)
  DVE : three bf16 mults (attn*h2 thirds) + two bf16 full-width adds into
        a per-4-SB staging tile; the bf16->f32 conversion and the 16ki->1
        halving tail run once per 4 SBs as strided batched f32 ops
        (stage_tail), writing f32r s_agg that m3 consumes directly
Dense phase (residual+LN+MLP+LN+mask, 4 chunks of 128 nodes) interleaves
with the main loop as each chunk's aggregates land. Both LN rstds are
computed on the DVE with the bit-trick + 1 Newton step so the ACT table
never leaves gelu_and_others (sqrt lives in another table; a switch costs
1283ns). All small constants are packed into two big host-assembled
tensors -> 2 dma_starts instead of 21 (each dma_start costs ~605ns
serialized on the sync queue's descriptor generator, which gated startup).
"""

import numpy as np
import ml_dtypes
from contextlib import ExitStack

import concourse.bacc as bacc
import concourse.tile as tile
from concourse import mybir
from concourse._compat import with_exitstack
from concourse.bass_utils import run_bass_kernel_spmd

F32 = mybir.dt.float32
BF16 = mybir.dt.bfloat16
I32 = mybir.dt.int32
GELU = mybir.ActivationFunctionType.Gelu
IDENT = mybir.ActivationFunctionType.Identity
SQUARE = mybir.ActivationFunctionType.Square
ADD = mybir.AluOpType.add
SUB = mybir.AluOpType.subtract
MULT = mybir.AluOpType.mult
XOR = mybir.AluOpType.bitwise_xor
SHR = mybir.AluOpType.logical_shift_right
AXX = mybir.AxisListType.X

NPBF = ml_dtypes.bfloat16

# Problem constants
N, K, C, ECTX, HID = 4096, 48, 128, 384, 512
NCORES = 8
NN = N // NCORES            # nodes per core = 512
R = NN * K                  # edge rows per core = 24576
SBN = 32                    # nodes per super-block
SBR = SBN * K               # rows per super-block = 1536
NSB = NN // SBN             # super-blocks per core = 16
EPS = 1e-5
SCALE = 30.0

# packed-constant column maps (columns in the two big const tensors)
_BF_COLS = [("w1e", 384), ("w1n", 128), ("w2", 128),
            ("wd2", 512), ("node_r", 512), ("row0", 640)]
_F32_COLS = [("node_t", 512), ("g1r", 128), ("be1r", 128), ("g2r", 128),
             ("be2r", 128), ("ident", 128), ("b1c", 1), ("b2c", 1),
             ("bd1", 4), ("bd2", 1), ("mask_t", 4), ("be2m", 512)]
_FR_COLS = [("w3", 128), ("wd1", 512)]
# row0 sub-layout (partition 0 of cbf): b3r at [0,128), sum_a at [128,640)


def _col_offsets(cols):
    off, out = 0, {}
    for name, w in cols:
        out[name] = (off, off + w)
        off += w
    return out, off


_BF_OFF, _BF_W = _col_offsets(_BF_COLS)
_F32_OFF, _F32_W = _col_offsets(_F32_COLS)
_FR_OFF, _FR_W = _col_offsets(_FR_COLS)


@with_exitstack
def _decoder_kernel(ctx: ExitStack, tc: tile.TileContext, aps: dict):
    nc = tc.nc

    consts = ctx.enter_context(tc.tile_pool(name="consts", bufs=1))
    # PSUM budget (8 banks of 2KB): pA 2 + pB 1 + ps2 3 + dense 2 = 8
    pAp = ctx.enter_context(tc.tile_pool(name="pAp", bufs=1, space="PSUM"))
    pBp = ctx.enter_context(tc.tile_pool(name="pBp", bufs=1, space="PSUM"))
    p2p = ctx.enter_context(tc.tile_pool(name="p2p", bufs=1, space="PSUM"))
    dpsp = ctx.enter_context(tc.tile_pool(name="dpsp", bufs=2, space="PSUM"))
    epool = ctx.enter_context(tc.tile_pool(name="epool", bufs=4))
    abpool = ctx.enter_context(tc.tile_pool(name="abpool", bufs=3))
    hpool = ctx.enter_context(tc.tile_pool(name="hpool", bufs=2))
    Ubpool = ctx.enter_context(tc.tile_pool(name="Ubpool", bufs=2))
    dpool = ctx.enter_context(tc.tile_pool(name="dpool", bufs=3))
    small = ctx.enter_context(tc.tile_pool(name="small", bufs=4))

    # ---- packed constants: 2 dma_starts; attention rows: 1 dma_start.
    # Order: cbf (m1 weights) -> edges(0) [below] -> cf32 -> attn; each
    # dma_start costs ~640ns of serialized sync-queue descriptor time, so
    # the ones gating the first matmul go first. ----
    cbf = consts.tile([128, _BF_W], BF16, tag="cbf")
    nc.sync.dma_start(cbf[:], aps["cbf"][:])
    cf32 = consts.tile([128, _F32_W], F32, tag="cf32")
    cfr = consts.tile([128, _FR_W], mybir.dt.float32r, tag="cfr")
    attn_all = consts.tile([1, R], BF16, tag="attn_all")

    def bfc(name):
        a, b = _BF_OFF[name]
        return cbf[:, a:b]

    def f32c(name):
        a, b = _F32_OFF[name]
        return cf32[:, a:b]

    w1e = bfc("w1e").rearrange("p (c f) -> p c f", c=3)
    w1n = bfc("w1n")
    w2 = bfc("w2")
    w3 = cfr[:, _FR_OFF["w3"][0]:_FR_OFF["w3"][1]]
    wd1 = cfr[:, _FR_OFF["wd1"][0]:_FR_OFF["wd1"][1]]
    wd2 = bfc("wd2").rearrange("p (j f) -> p j f", j=4)
    node_r = bfc("node_r")
    r0a, _ = _BF_OFF["row0"]
    b3r = cbf[0:1, r0a:r0a + 128]
    sum_a = cbf[0:1, r0a + 128:r0a + 640]
    node_t = f32c("node_t")
    g1r, be1r = f32c("g1r"), f32c("be1r")
    g2r, be2r = f32c("g2r"), f32c("be2r")
    ident = f32c("ident")
    b1c, b2c = f32c("b1c"), f32c("b2c")
    bd1, bd2 = f32c("bd1"), f32c("bd2")
    mask_t = f32c("mask_t")
    be2m = f32c("be2m")

    # aggregated masked message sums, f32r (pre-rounded for the m3
    # matmul), feature-major [C, nodes]
    s_agg = consts.tile([128, NN], mybir.dt.float32r, tag="s_agg")

    edges = aps["edges"]
    st = {}

    def dma_edges(t, split=False):
        eT = epool.tile([128, 3 * SBR], BF16, tag="eT")
        base = t * 3 * SBR
        if split:
            for c in range(3):
                nc.sync.dma_start(
                    eT[:, c * SBR:(c + 1) * SBR],
                    edges[:, base + c * SBR:base + (c + 1) * SBR])
        else:
            nc.sync.dma_start(eT[:], edges[:, base:base + 3 * SBR])
        st.setdefault(t, {})["eT"] = eT

    def make_atb(t):
        atb = abpool.tile([128, SBR], BF16, tag="atb")
        nc.gpsimd.partition_broadcast(
            atb[:], attn_all[0:1, t * SBR:(t + 1) * SBR])
        st.setdefault(t, {})["atb"] = atb

    def stageB(t):
        """m1 for super-block t. k-major columns: a 512-col group holds 16
        consecutive kk values for all 32 nodes, so the node-feature term is
        a stride-0 [128,16,32] view - identical for all three groups."""
        s_ = st[t]
        eT = s_["eT"]
        h1 = hpool.tile([128, SBR], BF16, tag="h1")
        nv = node_r[:, t * SBN:(t + 1) * SBN].unsqueeze(1) \
            .broadcast_to([128, 16, SBN])
        # half A: groups 0,1 -> pA [128,1024]; half B: group 2 -> pB
        pa = pAp.tile([128, 1024], F32, tag="pa")
        for c in range(3):
            for g in range(2):
                nc.tensor.matmul(
                    pa[:, g * 512:(g + 1) * 512], w1e[:, c, :],
                    eT[:, c * SBR + g * 512: c * SBR + (g + 1) * 512],
                    start=(c == 0), stop=False)
        for g in range(2):
            nc.tensor.matmul(
                pa[:, g * 512:(g + 1) * 512]
                .rearrange("p (kk n) -> p kk n", n=SBN),
                w1n[:], nv, start=False, stop=True)
        nc.scalar.activation(h1[:, 0:1024], pa[:], GELU, bias=b1c)
        pb = pBp.tile([128, 512], F32, tag="pb")
        for c in range(3):
            nc.tensor.matmul(
                pb[:], w1e[:, c, :],
                eT[:, c * SBR + 1024: c * SBR + 1536],
                start=(c == 0), stop=False)
        nc.tensor.matmul(
            pb[:].rearrange("p (kk n) -> p kk n", n=SBN),
            w1n[:], nv, start=False, stop=True)
        nc.scalar.activation(h1[:, 1024:1536], pb[:], GELU, bias=b1c)
        s_["h1"] = h1

    def stageC(t):
        s_ = st[t]
        h1 = s_["h1"]
        ps2 = p2p.tile([128, SBR], F32, tag="ps2")
        for s in range(3):
            nc.tensor.matmul(ps2[:, s * 512:(s + 1) * 512], w2[:],
                             h1[:, s * 512:(s + 1) * 512],
                             start=True, stop=True)
        h2 = hpool.tile([128, SBR], BF16, tag="h2")
        nc.scalar.activation(h2[:], ps2[:], GELU, bias=b2c)
        s_["h2"] = h2

    def stageD(t):
        """Masked k-aggregation. DVE bf16 packed ops are catastrophically
        slow (~+2us) when an operand's base offset is not 512B-aligned, so
        the tree only ever reads bf16 at 512B-multiple offsets from
        distinct tiles, then switches to fp32 (alignment-immune, and f32's
        4B elems keep 512B granularity down to 128 cols) for the 8->1
        tail. k-major layout keeps every run contiguous."""
        s_ = st[t]
        h2 = s_["h2"]
        atb = s_["atb"]
        # The only DVE op class never observed to hit the flat ~2us bf16
        # pathology: full-width read of tile X + full-width read of tile Y
        # -> output. So the mask-multiply runs as two halves (24 kk each)
        # and the whole per-SB k-collapse is ONE full-width add of the two
        # product tiles; the rest of the 24ki->1 tail is batched per 4 SBs
        # in fp32 (pathology-immune) by stage_tail.
        mlo = hpool.tile([128, 768], BF16, tag="mlo")
        mhi = hpool.tile([128, 768], BF16, tag="mhi")
        nc.vector.tensor_tensor(mlo[:], h2[:, 0:768], atb[:, 0:768],
                                op=MULT)
        nc.vector.tensor_tensor(mhi[:], h2[:, 768:1536], atb[:, 768:1536],
                                op=MULT)
        ub = st.setdefault(("U", t // 4), {}).setdefault(
            "U", Ubpool.tile([128, 3072], BF16, tag="Ub", name="ub"))
        nc.vector.tensor_tensor(ub[:, (t % 4) * 768:(t % 4 + 1) * 768],
                                mlo[:], mhi[:], op=ADD)
        del st[t]

    def stage_tail(g):
        """Batched bf16->f32 conversion + 16ki->1 tail for dense chunk g
        (4 super-blocks): one mode-exit instead of four, strided 4-row f32
        ops (f32 is immune to the packed-mode pathologies)."""
        ubd = st.pop(("U", g))["U"]
        uv = ubd[:].rearrange("p (t x) -> p t x", t=4)
        pf = hpool.tile([128, 1536], F32, tag="tr_pf")
        pv = pf[:].rearrange("p (t x) -> p t x", t=4)
        nc.vector.tensor_tensor(pv, uv[:, :, 0:384], uv[:, :, 384:768],
                                op=ADD)
        q = hpool.tile([128, 768], F32, tag="tr_q")
        qv = q[:].rearrange("p (t x) -> p t x", t=4)
        nc.vector.tensor_tensor(qv, pv[:, :, 0:192], pv[:, :, 192:384],
                                op=ADD)
        r = hpool.tile([128, 384], F32, tag="tr_r")
        rv = r[:].rearrange("p (t x) -> p t x", t=4)
        nc.vector.tensor_tensor(rv, qv[:, :, 0:96], qv[:, :, 96:192],
                                op=ADD)
        s1 = hpool.tile([128, 128], F32, tag="tr_s1")
        s1v = s1[:].rearrange("p (t x) -> p t x", t=4)
        nc.vector.tensor_tensor(s1v, rv[:, :, 0:32], rv[:, :, 32:64],
                                op=ADD)
        sv = s_agg[:, g * 128:(g + 1) * 128] \
            .rearrange("p (t x) -> p t x", t=4)
        nc.vector.tensor_tensor(sv, s1v, rv[:, :, 64:96], op=ADD)

    # ---- DVE rsqrt (bit trick + 1 Newton step, Lomont constant);
    # keeps sqrt off the ACT engine so its table never reloads ----
    def dve_rsqrt_g(v, dst):
        qi = small.tile([128, 1], I32, tag="qi")
        nc.vector.tensor_scalar(qi[:], v.bitcast(I32), 1, -1, op0=SHR,
                                op1=XOR)
        nc.vector.tensor_scalar(qi[:], qi[:], 0x5f375a87, None, op0=ADD)
        yield
        r = qi[:].bitcast(F32)
        t1 = small.tile([128, 1], F32, tag="t1")
        nc.vector.tensor_tensor(t1[:], v, r, op=MULT)
        nc.vector.tensor_tensor(t1[:], t1[:], r, op=MULT)
        yield
        nc.vector.tensor_scalar(t1[:], t1[:], -0.5, 1.5, op0=MULT, op1=ADD)
        nc.vector.tensor_tensor(dst, r, t1[:], op=MULT)
        yield

    def ln_chunk_g(x, g_rep, be_rep, out_t):
        """LayerNorm over the free dim of a row-major [128,128] f32 tile.
        Stats: DVE mean + ACT square-accum variance (square is in the gelu
        table); rstd on DVE. Generator."""
        mu = small.tile([128, 1], F32, tag="mu")
        nc.vector.tensor_reduce(mu[:], x[:], axis=AXX, op=ADD)
        mu_s = small.tile([128, 1], F32, tag="mu_s")
        nc.vector.tensor_scalar_mul(mu_s[:], mu[:], -1.0 / 128.0)
        yield
        xc = dpool.tile([128, 128], F32, tag="xc")
        nc.scalar.activation(xc[:], x[:], IDENT, bias=mu_s[:, :])
        yield
        sq = dpool.tile([128, 128], F32, tag="sq")
        vs = small.tile([128, 1], F32, tag="vs")
        nc.scalar.activation(sq[:], xc[:], SQUARE, accum_out=vs[:, :])
        yield
        vv = small.tile([128, 1], F32, tag="vv")
        nc.vector.tensor_scalar(vv[:], vs[:], 1.0 / 128.0, EPS, op0=MULT,
                                op1=ADD)
        rstd = small.tile([128, 1], F32, tag="rstd")
        yield from dve_rsqrt_g(vv[:], rstd[:])
        if be_rep is None:
            nc.vector.scalar_tensor_tensor(out_t[:], xc[:], rstd[:, :],
                                           g_rep, op0=MULT, op1=MULT)
        else:
            xg = dpool.tile([128, 128], F32, tag="xg")
            nc.vector.scalar_tensor_tensor(xg[:], xc[:], rstd[:, :], g_rep,
                                           op0=MULT, op1=MULT)
            nc.vector.tensor_tensor(out_t[:], xg[:], be_rep, op=ADD)
        yield

    def dense_chunk(ch):
        """Residual + LN1 + dense MLP + LN2 + mask for nodes
        [ch*128, (ch+1)*128). Generator; interleaved with the main loop."""
        sl = slice(ch * 128, (ch + 1) * 128)
        dps = dpsp.tile([128, 512], F32, tag="dps")
        _slot = [0]

        def dsub():
            v = dps[:, _slot[0] * 128:(_slot[0] + 1) * 128]
            _slot[0] = (_slot[0] + 1) % 4
            return v

        # agg = W3 @ s + b3 (x) sum_a, accumulated in one PSUM group
        # (s_agg fed to the PE directly as f32r: no cast, no DVE dep)
        dp0 = dsub()
        nc.tensor.matmul(dp0, w3, s_agg[:, sl], start=True, stop=False)
        nc.tensor.matmul(dp0, b3r, sum_a[:, sl], start=False, stop=True)
        xTb = dpool.tile([128, 128], F32, tag="xTb")
        nc.vector.tensor_tensor(xTb[:], node_t[:, sl], dp0, op=ADD)
        yield
        dp1 = dsub()
        nc.tensor.transpose(dp1, xTb[:], ident)
        x_rm = dpool.tile([128, 128], F32, tag="x_rm")
        nc.vector.tensor_copy(x_rm[:], dp1)
        yield
        x1n = dpool.tile([128, 128], F32, tag="x1n")
        yield from ln_chunk_g(x_rm, g1r, be1r, x1n)
        dp2 = dsub()
        nc.tensor.transpose(dp2, x1n[:], ident)
        x1nT = dpool.tile([128, 128], mybir.dt.float32r, tag="x1nT")
        nc.vector.tensor_copy(x1nT[:], dp2)
        yield
        hds = []
        for j in range(4):
            psd = dsub()
            nc.tensor.matmul(psd, wd1[:, j * 128:(j + 1) * 128], x1nT[:],
                             start=True, stop=True)
            h = dpool.tile([128, 128], BF16, tag=f"hd{j}")
            nc.scalar.activation(h[:], psd, GELU, bias=bd1[:, j:j + 1])
            hds.append(h)
            yield
        psd2 = dsub()
        for j in range(4):
            nc.tensor.matmul(psd2, wd2[:, j, :], hds[j][:],
                             start=(j == 0), stop=(j == 3))
        dT = dpool.tile([128, 128], F32, tag="dT")
        nc.scalar.activation(dT[:], psd2, IDENT, bias=bd2)
        yield
        dp4 = dsub()
        nc.tensor.transpose(dp4, dT[:], ident)
        x2 = dpool.tile([128, 128], F32, tag="x2")
        nc.vector.tensor_tensor(x2[:], x1n[:], dp4, op=ADD)
        yield
        x2n = dpool.tile([128, 128], F32, tag="x2n")
        yield from ln_chunk_g(x2, g2r, None, x2n)
        o_sb = dpool.tile([128, 128], F32, tag="o_sb")
        nc.vector.scalar_tensor_tensor(o_sb[:], x2n[:],
                                       mask_t[:, ch:ch + 1], be2m[:, sl],
                                       op0=MULT, op1=ADD)
        nc.sync.dma_start(aps["out"][sl, :], o_sb[:])

    # ---- pipelined emission ----
    dma_edges(0)
    nc.sync.dma_start(cf32[:], aps["cf32"][:])
    nc.sync.dma_start(cfr[:], aps["cfr"][:])
    dma_edges(1)
    nc.sync.dma_start(attn_all[:], aps["attn"][:])
    dma_edges(2)

    gens = []      # active dense-chunk generators
    pending = []   # chunks activated one iteration after their data lands

    def pump(steps):
        for _ in range(steps):
            for g in gens[:]:
                try:
                    next(g)
                except StopIteration:
                    gens.remove(g)

    for t in range(NSB + 2):
        if 1 <= t <= NSB:
            stageC(t - 1)                # PE m2(t-1) + ACT gelu2(t-1)
        if t < NSB:
            stageB(t)                    # PE m1(t) + ACT gelu1(t)
        if t < NSB:
            make_atb(t)                  # gpsimd bcast, 2 periods ahead
        if t >= 2:
            stageD(t - 2)                # DVE aggregation
            if (t - 2) % 4 == 3:
                stage_tail((t - 2) // 4)
                pending.append(dense_chunk((t - 2) // 4))
        if t + 3 < NSB:
            dma_edges(t + 3)
        pump(2)
        gens.extend(pending)
        pending.clear()
    while gens:
        pump(1)


_CACHE = {}


def _build_program():
    if "nc" in _CACHE:
        return _CACHE["nc"]
    nc = bacc.Bacc("TRN2", target_bir_lowering=False, debug=False)
    aps = {}

    def din(name, shape, dtype):
        aps[name] = nc.dram_tensor(name, shape, dtype,
                                   kind="ExternalInput").ap()

    din("edges", [128, NSB * 3 * SBR], BF16)
    din("attn", [1, R], BF16)
    din("cbf", [128, _BF_W], BF16)
    din("cf32", [128, _F32_W], F32)
    din("cfr", [128, _FR_W], mybir.dt.float32r)
    aps["out"] = nc.dram_tensor("out", [NN, C], F32,
                                kind="ExternalOutput").ap()

    with tile.TileContext(nc) as tc:
        _decoder_kernel(tc, aps)
    nc.compile()
    _CACHE["nc"] = nc
    return nc


def _prep_shared(W_m1, b_m1, W_m2, b_m2, W_m3, b_m3, g1, beta1,
                 W_d1, b_d1, W_d2, b_d2, g2, beta2):
    """Assemble the per-core-invariant packed constant tensors (minus the
    per-core rows: node_t/node_r/b3r/sum_a filled in _make_in_maps)."""
    f = np.float32
    rep = lambda v: np.tile(np.asarray(v, f)[None, :], (128, 1))

    cbf = np.zeros((128, _BF_W), dtype=NPBF)

    def put_bf(name, arr):
        a, b = _BF_OFF[name]
        cbf[:, a:b] = np.asarray(arr, f).astype(NPBF)

    put_bf("w1e", np.asarray(W_m1, f)[:, C:].T.reshape(3, 128, 128)
           .transpose(1, 0, 2).reshape(128, 384))
    put_bf("w1n", np.asarray(W_m1, f)[:, :C].T)
    put_bf("w2", np.asarray(W_m2, f).T)
    put_bf("wd2", np.asarray(W_d2, f).T.reshape(4, 128, 128)
           .transpose(1, 0, 2).reshape(128, 512))

    cf32 = np.zeros((128, _F32_W), dtype=f)
    cfr = np.zeros((128, _FR_W), dtype=f)

    def put_f32(name, arr):
        a, b = _F32_OFF[name]
        cf32[:, a:b] = np.asarray(arr, f).reshape(128, b - a)

    cfr[:, _FR_OFF["w3"][0]:_FR_OFF["w3"][1]] = (np.asarray(W_m3, f) / SCALE).T
    cfr[:, _FR_OFF["wd1"][0]:_FR_OFF["wd1"][1]] = np.asarray(W_d1, f).T
    put_f32("g1r", rep(g1)); put_f32("be1r", rep(beta1))
    put_f32("g2r", rep(g2)); put_f32("be2r", rep(beta2))
    put_f32("ident", np.eye(128, dtype=f))
    put_f32("b1c", np.asarray(b_m1, f)[:, None])
    put_f32("b2c", np.asarray(b_m2, f)[:, None])
    put_f32("bd1", np.asarray(b_d1, f).reshape(4, 128).T)
    put_f32("bd2", np.asarray(b_d2, f)[:, None])

    b3s = (np.asarray(b_m3, f) / SCALE).astype(NPBF)
    return {"cbf": cbf, "cf32": cf32, "cfr": cfr, "b3s": b3s,
            "beta2": np.asarray(beta2, f)}


def _make_in_maps(node_features, layer_edge_features, mask, attention_mask,
                  shared):
    f = np.float32
    node_features = np.asarray(node_features, f)
    mask = np.asarray(mask, f)
    attention_mask = np.asarray(attention_mask, f)
    lef16 = np.asarray(layer_edge_features, f).astype(NPBF)

    r0a, _ = _BF_OFF["row0"]
    nta, ntb = _F32_OFF["node_t"]
    ma, mb = _F32_OFF["mask_t"]
    ba, bb = _F32_OFF["be2m"]
    nra, nrb = _BF_OFF["node_r"]

    in_maps = []
    for ci in range(NCORES):
        lo, hi = ci * NN, (ci + 1) * NN
        # per-SB chunk-major, then k (neighbor), then node: [p, t, c, kk, n]
        a = lef16[lo:hi].reshape(NSB, SBN, K, 3, 128)
        edges_il = np.ascontiguousarray(
            a.transpose(4, 0, 3, 2, 1).reshape(128, NSB * 3 * SBR))
        am = attention_mask[lo:hi]
        attn_il = np.ascontiguousarray(
            am.reshape(NSB, SBN, K).transpose(0, 2, 1).reshape(1, R)
            .astype(NPBF))
        nt = np.ascontiguousarray(node_features[lo:hi].T)

        cbf = shared["cbf"].copy()
        cbf[:, nra:nrb] = nt.astype(NPBF)
        cbf[0, r0a:r0a + 128] = shared["b3s"]
        cbf[0, r0a + 128:r0a + 640] = am.sum(axis=1).astype(f).astype(NPBF)
        cf32 = shared["cf32"].copy()
        cf32[:, nta:ntb] = nt
        cf32[:, ma:mb] = mask[lo:hi].reshape(4, 128).T
        mmch = mask[lo:hi].reshape(4, 128).T
        cf32[:, ba:bb] = (mmch[:, :, None] *
                          shared["beta2"][None, None, :]).reshape(128, 512)

        in_maps.append({"edges": edges_il, "attn": attn_il, "cbf": cbf,
                        "cf32": cf32, "cfr": shared["cfr"]})
    return in_maps


def kernel(node_features, layer_edge_features, mask, attention_mask,
           W_m1, b_m1, W_m2, b_m2, W_m3, b_m3, g1, beta1,
           W_d1, b_d1, W_d2, b_d2, g2, beta2):
    shared = _prep_shared(W_m1, b_m1, W_m2, b_m2, W_m3, b_m3, g1, beta1,
                          W_d1, b_d1, W_d2, b_d2, g2, beta2)
    in_maps = _make_in_maps(node_features, layer_edge_features, mask,
                            attention_mask, shared)
    nc = _build_program()
    res = run_bass_kernel_spmd(nc, in_maps, core_ids=list(range(NCORES)))
    out = np.concatenate([res.results[i]["out"] for i in range(NCORES)],
                         axis=0)
    return out.astype(np.float32)
